# revision 9
# baseline (speedup 1.0000x reference)
"""Causal self-attention (B=4, T=2048, C=512, H=8, D=64) on 8 TRN2 NeuronCores.

Sharding: core = (batch b, head-group hg): 4 batches x 2 head groups of 4
heads.  Each core computes q/k/v projections for its 4 heads, causal
attention, and a partial output projection (its 256 rows of W_out); the host
sums the two head-group partials per batch.

v2 kernel layout notes:
  - x fed pre-transposed ([C, T]); S^T blocks ([tk, tq]) computed directly by
    operand swap, so no on-chip transposes anywhere.
  - Attention P/V use fp8e4m3 for supertiles s>=1: the exp is biased by -4.5
    (cancels in the softmax normalization) so P fits e4m3 range; supertile 0
    stays bf16 (tiny-tq rows can have all-tiny probabilities that would
    underflow fp8).  Full off-diagonal tk-block pairs go through fp8
    DoubleRow matmuls (2 tk blocks per instruction, 2x PE throughput); the 4
    ragged diagonal blocks per supertile are plain fp8 matmuls.
  - V is stored padded to 128 stationary columns (64 v + 1 ones + 63
    don't-care) because DoubleRow requires a full-width stationary; the
    don't-care output partitions are never read.
  - Softmax denominators ride along as the ones-column (psum row 64);
    normalization: DVE reciprocal straight off PSUM, gpsimd partition
    broadcast, DVE multiply on the O^T eviction.
  - Engine budget: ACT = exp + half the qk evictions; DVE = masks,
    reciprocal, normalization, yproj evictions; gpsimd = broadcast,
    v evictions, rest of qk evictions.
"""

import os
from contextlib import ExitStack

import numpy as np
import ml_dtypes

import concourse.bass as bass
import concourse.tile as tile
from concourse import bacc, mybir
from concourse.bass import ts, ds
from concourse.bass_utils import run_bass_kernel_spmd
from concourse.masks import make_upper_triangular

B = 4
T = 2048
C = 512
H = 8
D = 64
HG = 2                 # head groups (tensor-parallel dim)
HPC = H // HG          # heads per core = 4
M = HPC * D            # local head width = 256
P = 128
NT = T // P            # 16 t-tiles
NS = T // 512          # 4 t-supertiles
KC = C // P            # 4 contraction chunks of x
F32 = mybir.dt.float32
BF16 = mybir.dt.bfloat16
E4 = mybir.dt.float8e4
PBIAS = -4.5           # exp bias for fp8 supertiles (cancels in softmax)

_LAST_RESULTS = None


def build_attention_kernel():
    nc = bacc.Bacc("TRN2", target_bir_lowering=False, debug=False, num_devices=B * HG)

    xT = nc.dram_tensor("xT", [C, T], BF16, kind="ExternalInput").ap()
    wq = nc.dram_tensor("wq", [C, M], BF16, kind="ExternalInput").ap()
    wk = nc.dram_tensor("wk", [C, M], BF16, kind="ExternalInput").ap()
    wv = nc.dram_tensor("wv", [C, M], BF16, kind="ExternalInput").ap()
    wo = nc.dram_tensor("wo", [M, C], BF16, kind="ExternalInput").ap()
    y = nc.dram_tensor("y", [T, C], F32, kind="ExternalOutput").ap()

    with tile.TileContext(nc) as tc:
        with ExitStack() as ctx:
            emit_kernel(ctx, tc, xT, wq, wk, wv, wo, y)
    nc.compile()
    return nc


def emit_kernel(ctx, tc, xT, wq, wk, wv, wo, y):
    nc = tc.nc
    Exp = mybir.ActivationFunctionType.Exp
    scale = 1.0 / np.sqrt(D)

    const = ctx.enter_context(tc.tile_pool(name="const", bufs=1))
    xt_pool = ctx.enter_context(tc.tile_pool(name="xt", bufs=1))
    w_pool = ctx.enter_context(tc.tile_pool(name="w", bufs=1))
    qkv_pool = ctx.enter_context(tc.tile_pool(name="qkv", bufs=1))
    ptb_pool = ctx.enter_context(tc.tile_pool(name="ptb", bufs=4))
    pt8_pool = ctx.enter_context(tc.tile_pool(name="pt8", bufs=4))
    ptr_pool = ctx.enter_context(tc.tile_pool(name="ptr", bufs=4))
    ot_pool = ctx.enter_context(tc.tile_pool(name="ot", bufs=1))
    ysb_pool = ctx.enter_context(tc.tile_pool(name="ysb", bufs=3))
    small_pool = ctx.enter_context(tc.tile_pool(name="small", bufs=6))
    psum_s = ctx.enter_context(tc.tile_pool(name="psum_s", bufs=2, space="PSUM"))
    psum_ot = ctx.enter_context(tc.tile_pool(name="psum_ot", bufs=2, space="PSUM"))

    # --- constants: merged-hs triangular masks (keep tk <= tq) ---
    triu_f32 = const.tile([P, P], F32)
    make_upper_triangular(nc, triu_f32[:], val=1.0, diag=True)
    triu_bf = const.tile([P, 2, P], BF16)
    nc.vector.tensor_copy(triu_bf[:, 0, :], triu_f32[:])
    nc.vector.tensor_copy(triu_bf[:, 1, :], triu_f32[:])
    triu_f8 = const.tile([P, 2, P], E4)
    nc.vector.tensor_copy(triu_f8[:, 0, :], triu_f32[:])
    nc.vector.tensor_copy(triu_f8[:, 1, :], triu_f32[:])

    # --- PE clock pre-warm during the initial DMA wait ---
    pbias = const.tile([P, 1], F32)
    nc.gpsimd.memset(pbias[:], PBIAS)

    warm_in = const.tile([P, D], BF16)
    nc.gpsimd.memset(warm_in[:], 1.0)
    warm_ps = psum_s.tile([P, 2, 512], F32, name="s_ps")
    for i in range(48):
        nc.tensor.matmul(
            warm_ps[0:D, 0, 0:D], warm_in[:], warm_in[:], start=True, stop=True
        )

    # --- load weights and xT ---
    wq_sb = w_pool.tile([P, KC, M], BF16)
    nc.sync.dma_start(wq_sb[:], wq.rearrange("(kc p) m -> p kc m", p=P))
    wk_sb = w_pool.tile([P, KC, M], BF16)
    nc.gpsimd.dma_start(wk_sb[:], wk.rearrange("(kc p) m -> p kc m", p=P))

    xt_sb = []
    for kc in range(KC):
        t_ = xt_pool.tile([P, T], BF16, name=f"xt{kc}")
        xt_sb.append(t_)

    def load_xt(tc_):
        for kc in range(KC):
            eng = nc.sync if kc % 2 == 0 else nc.gpsimd
            eng.dma_start(
                xt_sb[kc][:, ts(tc_, 512)], xT[ts(kc, P), ts(tc_, 512)]
            )

    load_xt(0)
    wv_sb = w_pool.tile([P, KC, M], BF16)
    nc.sync.dma_start(wv_sb[:], wv.rearrange("(kc p) m -> p kc m", p=P))
    for tc_ in range(1, NS):
        load_xt(tc_)
    wo_sb = w_pool.tile([P, M // P, C], BF16)
    nc.gpsimd.dma_start(wo_sb[:], wo.rearrange("(kc p) n -> p kc n", p=P))

    # --- QKV SBUF tiles ---
    qt_sb = [qkv_pool.tile([P, T], BF16, name=f"qt{i}") for i in range(M // P)]
    kt_sb = [qkv_pool.tile([P, T], BF16, name=f"kt{i}") for i in range(M // P)]
    # fp8 V padded to 128 stationary cols: [0:64]=v, [64]=ones, [65:128] junk
    v_f8 = qkv_pool.tile([P, NT, HPC, P], E4)
    nc.gpsimd.memset(v_f8[:, :, :, D : D + 1], 1.0)
    # bf16 V (+ones) for supertile 0 (tk blocks 0..3 only)
    v_bf = qkv_pool.tile([P, 4, HPC, D + 1], BF16)
    nc.gpsimd.memset(v_bf[:, :, :, D : D + 1], 1.0)

    _qk_flip = [0]

    def emit_qk_pair(pp, mo, w_sb, dst):
        # two 512-wide t-supertiles share one psum tile; merged eviction
        s_ps = psum_s.tile([P, 2, 512], F32, name="s_ps")
        for kc in range(KC):
            for half in (0, 1):
                nc.tensor.matmul(
                    s_ps[:, half, :],
                    w_sb[:, kc, ts(mo, P)],
                    xt_sb[kc][:, ts(2 * pp + half, 512)],
                    start=(kc == 0),
                    stop=(kc == KC - 1),
                )
        out = dst[mo][:, ds(1024 * pp, 1024)].rearrange("p (a b) -> p a b", a=2)
        _qk_flip[0] ^= 1
        if _qk_flip[0]:
            nc.scalar.copy(out, s_ps[:])
        else:
            nc.vector.tensor_copy(out, s_ps[:])

    def emit_v_block(tt):
        s_ps = psum_s.tile([P, 2, 512], F32, name="s_ps")
        ps = s_ps[:, 0, 0:M]
        for kc in range(KC):
            nc.tensor.matmul(
                ps,
                xt_sb[kc][:, ts(tt, P)],
                wv_sb[:, kc, :],
                start=(kc == 0),
                stop=(kc == KC - 1),
            )
        nc.scalar.copy(
            v_f8[:, tt, :, 0:D], ps.rearrange("p (h d) -> p h d", d=D)
        )
        if tt < 4:
            nc.scalar.copy(
                v_bf[:, tt, :, 0:D], ps.rearrange("p (h d) -> p h d", d=D)
            )

    def proj_groups_a():
        groups = []
        for mo in range(M // P):
            for w_sb, dst in ((wq_sb, qt_sb), (wk_sb, kt_sb)):
                groups.append(lambda m=mo, w=w_sb, d=dst: emit_qk_pair(0, m, w, d))
        for tt in range(0, 4):
            groups.append(lambda t=tt: emit_v_block(t))
        return groups

    def proj_groups_b():
        groups = []
        for mo in range(M // P):
            for w_sb, dst in ((wq_sb, qt_sb), (wk_sb, kt_sb)):
                groups.append(lambda m=mo, w=w_sb, d=dst: emit_qk_pair(1, m, w, d))
        for tt in range(4, 8):
            groups.append(lambda t=tt: emit_v_block(t))
        return groups

    # --- attention ---
    ot_sb = [ot_pool.tile([P, T], BF16, name=f"ot{i}") for i in range(M // P)]

    _y_flip = [0]

    def emit_yproj(tt):
        s_ps = psum_s.tile([P, 2, 512], F32, name="s_ps")
        ps = s_ps[:, 0, :]
        for mo in range(M // P):
            nc.tensor.matmul(
                ps,
                ot_sb[mo][:, ts(tt, P)],
                wo_sb[:, mo, :],
                start=(mo == 0),
                stop=(mo == M // P - 1),
            )
        y_sb = ysb_pool.tile([P, C], F32)
        _y_flip[0] ^= 1
        if _y_flip[0]:
            nc.vector.tensor_copy(y_sb[:], ps)
        else:
            nc.scalar.copy(y_sb[:], ps)
        nc.sync.dma_start(y[ts(tt, P), :], y_sb[:])

    def emit_s_block(s, j, mo, n):
        # S^T block [tk=128, n tq cols] for head pair mo, both hs
        s_ps = psum_s.tile([P, 2, 512], F32, name="s_ps")
        off = 512 * s + (512 - n)
        for hs, po in ((0, 0), (1, D)):
            nc.tensor.matmul(
                s_ps[:, hs, 0:n],
                kt_sb[mo][ds(po, D), ts(j, P)],
                qt_sb[mo][ds(po, D), ds(off, n)],
                start=True,
                stop=True,
            )
        return s_ps

    for g in proj_groups_a():
        g()

    pending_yproj = []
    for s in range(NS):
        fp8 = s > 0
        bias = pbias[:] if fp8 else 0.0
        # background PE work sprinkled into this supertile's attention stream
        if s == 0:
            bg = proj_groups_b()
        elif s == 1:
            bg = [lambda t=tt: emit_v_block(t) for tt in range(8, 16)]
            bg += [lambda t=tt: emit_yproj(t) for tt in pending_yproj]
        else:
            bg = [lambda t=tt: emit_yproj(t) for tt in pending_yproj]
        pending_yproj = list(range(4 * s, 4 * s + 4))

        # PE/ACT work units this supertile: per mo: 2s full pairs + 4 raggeds
        nunits = (2 * s + 4) * 2
        bg_every = max(1, (nunits + len(bg) - 1) // max(1, len(bg))) if bg else 0
        slot = 0

        ot_units = [
            psum_ot.tile([P, 2, 512], F32, name="ot_ps") for _ in range(M // P)
        ]

        def bg_tick():
            nonlocal slot
            if bg and bg_every and slot % bg_every == bg_every - 1 and bg:
                bg.pop(0)()
            slot += 1

        for mo in range(M // P):
            if fp8:
                # full off-diagonal tk blocks, DoubleRow pairs
                for jp in range(2 * s):
                    # layout [P, hs, jp, 512] so the DoubleRow rhs pair
                    # (dim jp) is contiguous per hs slice
                    pt8 = pt8_pool.tile([P, 2, 2, 512], E4)
                    for ji in (0, 1):
                        s_ps = emit_s_block(s, 2 * jp + ji, mo, 512)
                        nc.scalar.activation(
                            pt8[:, :, ji, :], s_ps[:], Exp, scale=scale, bias=bias
                        )
                    for hs, h in ((0, 2 * mo), (1, 2 * mo + 1)):
                        nc.tensor.matmul(
                            ot_units[mo][:, hs, :],
                            v_f8[:, ds(2 * jp, 2), h, :],
                            pt8[:, hs, :, :],
                            start=(jp == 0),
                            stop=False,
                            perf_mode=mybir.MatmulPerfMode.DoubleRow,
                        )
                    bg_tick()
            # ragged diagonal blocks (all 4 j of the diagonal supertile)
            for kk in range(4):
                j = 4 * s + kk
                off = P * kk
                n = 512 - off
                s_ps = emit_s_block(s, j, mo, n)
                if fp8:
                    ptr = ptr_pool.tile([P, 2, 512], E4)
                    tri = triu_f8
                else:
                    ptr = ptb_pool.tile([P, 2, 512], BF16)
                    tri = triu_bf
                nc.scalar.activation(
                    ptr[:, :, 0:n], s_ps[:, :, 0:n], Exp, scale=scale, bias=bias
                )
                nc.vector.tensor_mul(ptr[:, :, 0:P], ptr[:, :, 0:P], tri[:])
                for hs, h in ((0, 2 * mo), (1, 2 * mo + 1)):
                    if fp8:
                        lhsT = v_f8[:, j, h, :]
                        out = ot_units[mo][:, hs, ds(off, n)]
                    else:
                        lhsT = v_bf[:, j, h, :]
                        out = ot_units[mo][0 : D + 1, hs, ds(off, n)]
                    nc.tensor.matmul(
                        out,
                        lhsT,
                        ptr[:, hs, 0:n],
                        start=(not fp8 and kk == 0),
                        stop=(kk == 3),
                    )
                bg_tick()
        for g in bg:
            g()

        # normalization: recip straight off the psum ones-row, broadcast, mul
        for mo in range(M // P):
            ot_ps = ot_units[mo]
            sums = small_pool.tile([1, 2, 512], F32)
            if mo == 0:
                nc.scalar.copy(sums[:], ot_ps[ds(D, 1), :, :])
            else:
                nc.vector.tensor_copy(sums[:], ot_ps[ds(D, 1), :, :])
            recip = small_pool.tile([1, 2, 512], F32)
            nc.vector.reciprocal_approx_fast(recip[:], sums[:])
            bcast = small_pool.tile([D, 2, 512], F32)
            nc.gpsimd.partition_broadcast(bcast[:], recip[:])
            for hs, po in ((0, 0), (1, D)):
                nc.vector.tensor_mul(
                    ot_sb[mo][ds(po, D), ts(s, 512)],
                    ot_ps[0:D, hs, :],
                    bcast[:, hs, :],
                )

    for tt in pending_yproj:
        emit_yproj(tt)


def shard_inputs(x, W_qkv, W_out):
    """Full inputs -> list of 8 per-core input dicts (core = b*HG + hg)."""
    bf16 = ml_dtypes.bfloat16
    x = np.asarray(x, dtype=np.float32)
    W_qkv = np.asarray(W_qkv, dtype=np.float32).astype(bf16)
    W_out = np.asarray(W_out, dtype=np.float32).astype(bf16)
    in_maps = []
    for b in range(B):
        xT = np.ascontiguousarray(x[b].T).astype(bf16)
        for hg in range(HG):
            cols = slice(hg * M, (hg + 1) * M)
            in_maps.append(
                {
                    "xT": xT,
                    "wq": np.ascontiguousarray(W_qkv[:, 0 * C :][:, cols]),
                    "wk": np.ascontiguousarray(W_qkv[:, 1 * C :][:, cols]),
                    "wv": np.ascontiguousarray(W_qkv[:, 2 * C :][:, cols]),
                    "wo": np.ascontiguousarray(W_out[hg * M : (hg + 1) * M, :]),
                }
            )
    return in_maps


_NC_CACHE = None


def kernel(x, W_qkv, W_out):
    global _NC_CACHE, _LAST_RESULTS
    if _NC_CACHE is None:
        _NC_CACHE = build_attention_kernel()
    nc = _NC_CACHE
    in_maps = shard_inputs(x, W_qkv, W_out)
    kwargs = {}
    if os.environ.get("BASS_KERNEL_TRACE"):
        kwargs = dict(trace=True, tmpdir=os.environ.get("BASS_KERNEL_TRACE_DIR"))
    res = run_bass_kernel_spmd(nc, in_maps, core_ids=list(range(B * HG)), **kwargs)
    _LAST_RESULTS = res
    out = np.empty((B, T, C), dtype=np.float32)
    for b in range(B):
        out[b] = res.results[b * HG]["y"] + res.results[b * HG + 1]["y"]
    return out
